# revision 1
# baseline (speedup 1.0000x reference)
"""GPTQ int4 dequant + matmul kernel for Trainium2, column-parallel over 8 cores.

Computes out = x @ dequant(qweight, qzeros, scales) + bias where
  qweight: [OC//8, IC_total] int32 (nibbles packed along OC rows)
  qzeros:  [G, IC_total//8]  int32 (nibbles packed along IC cols)
  scales:  [G, IC_total]     float32
  x:       [N, OC]           float32
  bias:    [IC_total]        float32
Sharding: IC (out_features) split across 8 cores; x replicated.

Per-core kernel structure:
  1. zp unpack (strided shift/mask) + bias-bit trick (|0x4B000000 so the int
     nibble bits are exactly the fp32 value 2^23+zp) -> PE-transpose to
     [IC, G] layout so zp/s become per-partition scalars.
  2. qweight: DMA -> PE-transpose (int32, bit-exact permutation) to
     [IC, OC//8] layout; unpack nibbles with immediate shifts (strided
     free-dim writes); OR 0x4B000000; one fused tensor_scalar per group:
     W^T = ((2^23+nib) - (2^23+zp)) * s  -> bf16.  All bit-exact int ops +
     exact float ops; single rounding to bf16.
  3. dma_start_transpose W^T -> W [OC part, IC free] (bf16, xbar).
  4. Main loop over 128-row token tiles: gpsimd cast-DMA x (fp32->bf16),
     dma_start_transpose -> xT tiles; matmul with xT stationary, W streamed
     from SBUF, fp32 psum accumulation over OC; bias added via a K=1 matmul
     with a ones row; ACT drains psum -> SBUF; DMA out.
"""

import sys

if "/opt/trn_rl_repo" not in sys.path:
    sys.path.insert(0, "/opt/trn_rl_repo")

from contextlib import ExitStack

import numpy as np
import ml_dtypes

from concourse import bacc, bass, mybir, tile

P = 128
PACK = 8
FP32_BIAS_BITS = 0x4B000000  # fp32 bit pattern of 2**23
FP32_BIAS = float(2**23)

f32 = mybir.dt.float32
bf16 = mybir.dt.bfloat16
i32 = mybir.dt.int32
Alu = mybir.AluOpType

# Full problem dims (hardcoded per harness contract)
N_FULL = 4096
K_FULL = 4096  # OC / in_features (contraction)
IC_TOTAL = 11008
G_FULL = 32
N_CORES = 8
IC_SHARD = IC_TOTAL // N_CORES  # 1376


def _jtiles(ic):
    """IC j-tiles of <=128, last may be ragged (must stay %16 for xbar)."""
    tiles = []
    off = 0
    while off < ic:
        w = min(P, ic - off)
        assert w % 16 == 0, f"ragged j-tile {w} not multiple of 16"
        tiles.append((off, w))
        off += ic and w
    return tiles


def _chunks(ic):
    """Greedy grouping of j-tiles into psum chunks of <=512 fp32."""
    chunks = []
    start = 0
    for off, w in _jtiles(ic):
        if off + w - start > 512:
            chunks.append((start, off - start))
            start = off
    chunks.append((start, ic - start))
    return chunks


def build(nc, n=N_FULL, k=K_FULL, ic=IC_SHARD, g=G_FULL):
    """Emit the per-core program. All cores run the same program (SPMD)."""
    assert k % P == 0 and n % P == 0 and k // g == P
    KT = k // P  # contraction tiles (each == one quant group)
    NT = n // P  # token tiles
    jts = _jtiles(ic)
    chunks = _chunks(ic)
    # map j-tile -> (chunk index, offset within chunk)
    jt_chunk = []
    for off, w in jts:
        for ci, (c0, cw) in enumerate(chunks):
            if c0 <= off < c0 + cw:
                jt_chunk.append((ci, off - c0))
                break

    q_d = nc.dram_tensor("qweight", [k // PACK, ic], i32, kind="ExternalInput")
    qz_d = nc.dram_tensor("qzeros", [g, ic // PACK], i32, kind="ExternalInput")
    s_d = nc.dram_tensor("scales", [g, ic], f32, kind="ExternalInput")
    x_d = nc.dram_tensor("x", [n, k], f32, kind="ExternalInput")
    b_d = nc.dram_tensor("bias", [ic], f32, kind="ExternalInput")
    id128_d = nc.dram_tensor("id128_f32", [P, P], f32, kind="ExternalInput")
    idg_f_d = nc.dram_tensor("idg_f32", [g, g], f32, kind="ExternalInput")
    ones_d = nc.dram_tensor("ones_row", [1, P], bf16, kind="ExternalInput")
    out_d = nc.dram_tensor("out", [n, ic], f32, kind="ExternalOutput")

    with tile.TileContext(nc) as tc, ExitStack() as ctx:
        const = ctx.enter_context(tc.tile_pool(name="const", bufs=1))
        wpool = ctx.enter_context(tc.tile_pool(name="w", bufs=1))
        prep = ctx.enter_context(tc.tile_pool(name="prep", bufs=2))
        prep1 = ctx.enter_context(tc.tile_pool(name="prep1", bufs=1))
        xpool = ctx.enter_context(tc.tile_pool(name="x", bufs=2))
        opool = ctx.enter_context(tc.tile_pool(name="o", bufs=2))
        psum = ctx.enter_context(tc.tile_pool(name="psum", bufs=2, space="PSUM"))
        psum_t = ctx.enter_context(tc.tile_pool(name="psum_t", bufs=2, space="PSUM"))

        # ---- constants
        id128 = const.tile([P, P], f32)
        nc.sync.dma_start(out=id128[:], in_=id128_d[:])
        idg_f = const.tile([g, g], f32)
        nc.sync.dma_start(out=idg_f[:], in_=idg_f_d[:])
        ones = const.tile([1, P], bf16)
        nc.sync.dma_start(out=ones[:], in_=ones_d[:])
        bias_row = const.tile([1, ic], bf16)
        nc.gpsimd.dma_start(out=bias_row[:], in_=b_d[None, :])  # cast f32->bf16

        # ---- zp unpack: qzeros [g, ic//8] -> zp_or [g, ic] (bits = fp32 2^23+zp)
        qz_sb = const.tile([g, ic // PACK], i32)
        nc.sync.dma_start(out=qz_sb[:], in_=qz_d[:])
        zp_or = const.tile([g, ic], i32)
        for r in range(PACK):
            nc.vector.tensor_scalar(
                out=zp_or[:, r::PACK],
                in0=qz_sb[:],
                scalar1=4 * r,
                scalar2=15,
                op0=Alu.logical_shift_right,
                op1=Alu.bitwise_and,
            )
        nc.vector.tensor_scalar(
            out=zp_or[:], in0=zp_or[:], scalar1=FP32_BIAS_BITS, scalar2=None,
            op0=Alu.bitwise_or,
        )
        s_sb = const.tile([g, ic], f32)
        nc.sync.dma_start(out=s_sb[:], in_=s_d[:])

        # ---- transpose zp_or and scales to [IC-part, g] layout
        NJ = len(jts)
        zpT = const.tile([P, NJ, g], f32)  # bits are fp32 2^23+zp already
        sT = const.tile([P, NJ, g], f32)
        for ji, (off, w) in enumerate(jts):
            pz = psum_t.tile([P, P], f32, name="pst_f")
            nc.tensor.transpose(
                pz[:w, :g], zp_or.bitcast(f32)[:, off : off + w], idg_f[:]
            )
            nc.vector.tensor_copy(zpT[:w, ji, :], pz[:w, :g])
            ps_ = psum_t.tile([P, P], f32, name="pst_f")
            nc.tensor.transpose(ps_[:w, :g], s_sb[:, off : off + w], idg_f[:])
            nc.vector.tensor_copy(sT[:w, ji, :], ps_[:w, :g])

        # ---- W chunks in [OC-part, KT, chunk-width] bf16
        wtiles = [wpool.tile([P, KT, cw], bf16, name=f"Wc{ci}")
                  for ci, (c0, cw) in enumerate(chunks)]

        RP = k // PACK  # packed qweight rows
        rts = [(r0, min(P, RP - r0)) for r0 in range(0, RP, P)]
        for ji, (off, w) in enumerate(jts):
            # load qweight columns [off:off+w] as [<=128, n_rt, w]
            qw4 = prep.tile([P, len(rts), P], i32, name="qw4")
            for rt, (r0, rw) in enumerate(rts):
                nc.sync.dma_start(
                    out=qw4[:rw, rt, :w],
                    in_=q_d[r0 : r0 + rw, off : off + w],
                )
            # PE-transpose (bit-exact) -> qwT [w, k//8 packed rows]
            qwT = prep.tile([P, RP], i32, name="qwT")
            for rt, (r0, rw) in enumerate(rts):
                pq = psum_t.tile([P, P], f32, name="pst_f")
                nc.tensor.transpose(
                    pq[:w, :rw], qw4.bitcast(f32)[:rw, rt, :w], id128[:rw, :rw]
                )
                nc.vector.tensor_copy(qwT.bitcast(f32)[:w, r0 : r0 + rw], pq[:w, :rw])
            qwT_flat = qwT[:w, :]

            # unpack nibbles: nib[j, 8r+kk] = (qwT[j, r] >> 4kk) & 15
            nib = prep.tile([P, k], i32, name="nib")
            for kk in range(PACK):
                nc.vector.tensor_scalar(
                    out=nib[:w, kk::PACK],
                    in0=qwT_flat,
                    scalar1=4 * kk,
                    scalar2=15,
                    op0=Alu.logical_shift_right,
                    op1=Alu.bitwise_and,
                )
            nc.vector.tensor_scalar(
                out=nib[:w, :], in0=nib[:w, :], scalar1=FP32_BIAS_BITS,
                scalar2=None, op0=Alu.bitwise_or,
            )
            # dequant: WT = ((2^23+nib) - (2^23+zp)) * s -> bf16
            wt = prep.tile([P, k], bf16, name="wt")
            nibf = nib.bitcast(f32)
            for gi in range(g):
                nc.vector.tensor_scalar(
                    out=wt[:w, gi * P : (gi + 1) * P],
                    in0=nibf[:w, gi * P : (gi + 1) * P],
                    scalar1=zpT[:w, ji, gi : gi + 1],
                    scalar2=sT[:w, ji, gi : gi + 1],
                    op0=Alu.subtract,
                    op1=Alu.mult,
                )
            # xbar transpose WT [w, k] -> W [OC-part, KT, j-slice]
            ci, coff = jt_chunk[ji]
            nc.sync.dma_start_transpose(
                out=wtiles[ci][:, :, coff : coff + w], in_=wt[:w, :]
            )

        # ---- main loop over token tiles
        for nt in range(NT):
            xb = xpool.tile([P, k], bf16, name="xb")
            nc.gpsimd.dma_start(out=xb[:], in_=x_d[nt * P : (nt + 1) * P, :])
            xT = xpool.tile([P, KT, P], bf16, name="xT")
            nc.sync.dma_start_transpose(out=xT[:], in_=xb[:])

            ps = psum.tile([P, ic], f32, name="ps")
            for kt in range(KT):
                for ci, (c0, cw) in enumerate(chunks):
                    nc.tensor.matmul(
                        ps[:, c0 : c0 + cw],
                        lhsT=xT[:, kt, :],
                        rhs=wtiles[ci][:, kt, :],
                        start=(kt == 0),
                        stop=False,
                    )
            # bias via K=1 matmul with ones row (also closes the accum group)
            for ci, (c0, cw) in enumerate(chunks):
                nc.tensor.matmul(
                    ps[:, c0 : c0 + cw],
                    lhsT=ones[:, :],
                    rhs=bias_row[:, c0 : c0 + cw],
                    start=False,
                    stop=True,
                )
            out_sb = opool.tile([P, ic], f32, name="out_sb")
            nc.scalar.copy(out=out_sb[:], in_=ps[:])
            nc.sync.dma_start(
                out=out_d[nt * P : (nt + 1) * P, :], in_=out_sb[:]
            )
    return nc


def make_const_inputs(g=G_FULL):
    return {
        "id128_f32": np.eye(P, dtype=np.float32),
        "idg_f32": np.eye(g, dtype=np.float32),
        "ones_row": np.ones((1, P), dtype=ml_dtypes.bfloat16),
    }


def kernel(input, qweight, qzeros, scales, bias):
    """Full-problem entry point: shard, run on 8 cores, gather."""
    from concourse.bass_utils import run_bass_kernel_spmd

    nc = bacc.Bacc("TRN2", target_bir_lowering=False, debug=False)
    build(nc)
    nc.compile()

    consts = make_const_inputs()
    x = np.ascontiguousarray(input, dtype=np.float32)
    in_maps = []
    for c in range(N_CORES):
        j0, j1 = c * IC_SHARD, (c + 1) * IC_SHARD
        in_maps.append(
            {
                "qweight": np.ascontiguousarray(qweight[:, j0:j1]),
                "qzeros": np.ascontiguousarray(
                    qzeros[:, c * (IC_SHARD // PACK) : (c + 1) * (IC_SHARD // PACK)]
                ),
                "scales": np.ascontiguousarray(scales[:, j0:j1]),
                "x": x,
                "bias": np.ascontiguousarray(bias[j0:j1]),
                **consts,
            }
        )
    res = run_bass_kernel_spmd(nc, in_maps, list(range(N_CORES)))
    outs = [np.asarray(res.results[c]["out"], dtype=np.float32) for c in range(N_CORES)]
    return np.concatenate(outs, axis=1)



# revision 4
# speedup vs baseline: 1.1376x; 1.1376x over previous
"""GPTQ int4 dequant + matmul kernel for Trainium2, column-parallel over 8 cores.

Computes out = x @ dequant(qweight, qzeros, scales) + bias where
  qweight: [OC//8, IC_total] int32 (nibbles packed along OC rows)
  qzeros:  [G, IC_total//8]  int32 (nibbles packed along IC cols)
  scales:  [G, IC_total]     float32
  x:       [N, OC]           float32
  bias:    [IC_total]        float32
Sharding: IC (out_features) split across 8 cores; x replicated.

v2 design (zero-point-free weight path):
  W_deq[oc, j] = (Wq[oc,j] - zp[g,j]) * s[g,j]
  x @ W_deq    = x @ (Wq * s)  -  xg @ (zp * s)      (rank-G correction)
  where xg[n, g] = sum of x[n, k] over group g (128 contiguous k).

  Weight prep per core: qweight loads as 4 tiles [128 rp, ic] int32 with the
  contraction axis already on partitions (packed).  k-tile kt' = rt*8+kk holds
  original rows oc = (rt*128+rp)*8+kk -- a k-permutation absorbed by the x
  transpose DMA access pattern.  Per kt': one tensor_scalar (shift+and) to
  int32 nibbles + one tensor_tensor mult (int32 x fp32 -> bf16, exact int
  convert) against host-expanded scales sx[rt][p, j] = s[rt*8 + p//16, j].
  No PE transposes, no xbar W transpose, no OR/sub passes.

  Main loop per 128-token tile: gpsimd cast-DMA x->bf16, one transpose DMA
  with permuted src AP -> xT' [128, 32, 128]; DVE segmented reduce builds
  xg; PE-transpose -> corrT [33, 128] (ones row appended); 32 kt' matmuls
  over 3 psum chunks + 1 rank-33 correction matmul per chunk (adds
  -xg@(zp*s) and +bias, closes accumulation); ACT drains; DMA out.
"""

import sys

if "/opt/trn_rl_repo" not in sys.path:
    sys.path.insert(0, "/opt/trn_rl_repo")

from contextlib import ExitStack

import numpy as np
import ml_dtypes

from concourse import bacc, bass, mybir, tile

P = 128
PACK = 8

f32 = mybir.dt.float32
bf16 = mybir.dt.bfloat16
i32 = mybir.dt.int32
Alu = mybir.AluOpType

# Full problem dims (hardcoded per harness contract)
N_FULL = 4096
K_FULL = 4096  # OC / in_features (contraction)
IC_TOTAL = 11008
G_FULL = 32
N_CORES = 8
IC_SHARD = IC_TOTAL // N_CORES  # 1376
RT = K_FULL // PACK // P  # 4 packed-row tiles
KT = K_FULL // P  # 32 contraction tiles
NT = N_FULL // P  # 32 token tiles
CHUNKS = [(0, 512), (512, 512), (1024, IC_SHARD - 1024)]


def build(nc, n=N_FULL, k=K_FULL, ic=IC_SHARD, g=G_FULL):
    """Emit the per-core program. All cores run the same program (SPMD)."""
    q_d = nc.dram_tensor("qweight", [k // PACK, ic], i32, kind="ExternalInput")
    sx_d = nc.dram_tensor("sx", [k // PACK, ic], f32, kind="ExternalInput")
    z2b_d = nc.dram_tensor("z2b", [g + 1, ic], bf16, kind="ExternalInput")
    x_d = nc.dram_tensor("x", [n, k], f32, kind="ExternalInput")
    id128_d = nc.dram_tensor("id128_f32", [P, P], f32, kind="ExternalInput")
    ones_d = nc.dram_tensor("ones_row", [1, P], bf16, kind="ExternalInput")
    out_d = nc.dram_tensor("out", [n, ic], f32, kind="ExternalOutput")

    with tile.TileContext(nc) as tc, ExitStack() as ctx:
        const = ctx.enter_context(tc.tile_pool(name="const", bufs=1))
        sxpool = ctx.enter_context(tc.tile_pool(name="sx", bufs=1))
        wpool = ctx.enter_context(tc.tile_pool(name="w", bufs=1))
        qwpool = ctx.enter_context(tc.tile_pool(name="qw", bufs=2))
        nibpool = ctx.enter_context(tc.tile_pool(name="nib", bufs=2))
        xbpool = ctx.enter_context(tc.tile_pool(name="xb", bufs=2))
        xtpool = ctx.enter_context(tc.tile_pool(name="xt", bufs=3))
        xgpool = ctx.enter_context(tc.tile_pool(name="xg", bufs=2))
        corrpool = ctx.enter_context(tc.tile_pool(name="corr", bufs=2))
        opool = ctx.enter_context(tc.tile_pool(name="o", bufs=2))
        psum = ctx.enter_context(tc.tile_pool(name="psum", bufs=2, space="PSUM"))
        psum_t = ctx.enter_context(tc.tile_pool(name="psum_t", bufs=2, space="PSUM"))

        # ---- constants
        id128 = const.tile([P, P], f32)
        nc.sync.dma_start(out=id128[:], in_=id128_d[:])
        ones = const.tile([1, P], bf16)
        nc.sync.dma_start(out=ones[:], in_=ones_d[:])
        z2b = const.tile([g + 1, ic], bf16)
        nc.sync.dma_start(out=z2b[:], in_=z2b_d[:])
        sx = []
        for rt in range(RT):
            t = sxpool.tile([P, ic], f32, name=f"sx{rt}")
            nc.sync.dma_start(out=t[:], in_=sx_d[rt * P : (rt + 1) * P, :])
            sx.append(t)

        wtiles = [wpool.tile([P, ic], bf16, name=f"W{kt}") for kt in range(KT)]

        # ---- per-token-tile x pipeline (emitted for nt, consumed by main loop)
        xts = [None] * NT
        corrs = [None] * NT

        def emit_x(nt):
            # x columns are host-permuted: xb[:, kt*128 + rp] = x[:, (rt*128+rp)*8+kk]
            # for kt = rt*8+kk, so the plain transpose gives permuted k-tiles.
            xb = xbpool.tile([P, k], bf16, name="xb")
            nc.gpsimd.dma_start(out=xb[:], in_=x_d[nt * P : (nt + 1) * P, :])
            xt = xtpool.tile([P, KT, P], bf16, name="xT")
            nc.sync.dma_start_transpose(out=xt[:], in_=xb[:])
            xts[nt] = xt
            # group sums over original groups g = rt*8 + rp//16, two-stage:
            # stage 1: sum 16-runs (fixed rt, kk, q) -> xp [128, 256]
            xp = xgpool.tile([P, 256], f32, name="xp")
            nc.vector.tensor_reduce(
                out=xp[:],
                in_=xb[:, :].rearrange("p (m i) -> p m i", i=16),
                axis=mybir.AxisListType.X,
                op=Alu.add,
            )
            # stage 2: sum over kk: xg[t, rt*8+q] = sum_kk xp[t, (rt*8+kk)*8+q]
            xg = xgpool.tile([P, g], f32, name="xg")
            nc.vector.tensor_reduce(
                out=xg[:],
                in_=xp[:, :].rearrange("p (rt kk q) -> p rt q kk", rt=RT, kk=PACK),
                axis=mybir.AxisListType.X,
                op=Alu.add,
            )
            # transpose -> [32 g, 128 tok], stack ones row -> corrT [33, 128]
            pg = psum_t.tile([g, P], f32, name="pst")
            nc.tensor.transpose(pg[:, :], xg[:, :], id128[:])
            corrT = corrpool.tile([g + 1, P], bf16, name="corrT")
            nc.scalar.copy(out=corrT[0:g, :], in_=pg[:, :])
            nc.scalar.copy(out=corrT[g : g + 1, :], in_=ones[:])
            corrs[nt] = corrT

        emit_x(0)
        emit_x(1)

        # ---- weight prep: per rt, 8 nibble planes
        def emit_prep(rt_list):
            for rt in rt_list:
                qw = qwpool.tile([P, ic], i32, name="qw")
                nc.sync.dma_start(out=qw[:], in_=q_d[rt * P : (rt + 1) * P, :])
                for kk in range(PACK):
                    kt = rt * PACK + kk
                    nib = nibpool.tile([P, ic], i32, name="nib")
                    nc.vector.tensor_scalar(
                        out=nib[:],
                        in0=qw[:],
                        scalar1=4 * kk,
                        scalar2=15,
                        op0=Alu.logical_shift_right,
                        op1=Alu.bitwise_and,
                    )
                    nc.vector.tensor_tensor(
                        out=wtiles[kt][:], in0=nib[:], in1=sx[rt][:], op=Alu.mult
                    )

        emit_prep([0, 1])
        emit_x(2)
        emit_prep([2, 3])

        # ---- main loop over token tiles
        for nt in range(NT):
            if nt >= 3:
                emit_x(nt)
            xt = xts[nt]
            ps = psum.tile([P, ic], f32, name="ps")
            for kt in range(KT):
                for c0, cw in CHUNKS:
                    nc.tensor.matmul(
                        ps[:, c0 : c0 + cw],
                        lhsT=xt[:, kt, :],
                        rhs=wtiles[kt][:, c0 : c0 + cw],
                        start=(kt == 0),
                        stop=False,
                    )
            # rank-33 correction: adds -xg@(zp*s) and +bias, closes accumulation
            for c0, cw in CHUNKS:
                nc.tensor.matmul(
                    ps[:, c0 : c0 + cw],
                    lhsT=corrs[nt][:, :],
                    rhs=z2b[:, c0 : c0 + cw],
                    start=False,
                    stop=True,
                )
            out_sb = opool.tile([P, ic], f32, name="out_sb")
            nc.scalar.copy(out=out_sb[:], in_=ps[:])
            nc.sync.dma_start(out=out_d[nt * P : (nt + 1) * P, :], in_=out_sb[:])
    return nc


def make_const_inputs():
    return {
        "id128_f32": np.eye(P, dtype=np.float32),
        "ones_row": np.ones((1, P), dtype=ml_dtypes.bfloat16),
    }


def make_in_maps(input, qweight, qzeros, scales, bias):
    """Shard + host-side layout prep (scale expansion, zp*s table, x perm)."""
    consts = make_const_inputs()
    # permute x columns so k-tile kt'=rt*8+kk holds rows oc=(rt*128+rp)*8+kk
    rt = np.arange(RT)[:, None, None]
    kk = np.arange(PACK)[None, :, None]
    rp = np.arange(P)[None, None, :]
    perm = ((rt * P + rp) * PACK + kk).reshape(-1)  # [ (rt,kk,rp) -> oc ]
    x = np.ascontiguousarray(np.asarray(input, dtype=np.float32)[:, perm])
    # unpack qzeros -> zp [G, IC_TOTAL]
    col = np.arange(IC_TOTAL, dtype=np.int32)
    zp = (qzeros[:, col // PACK] >> ((col % PACK) * 4)[None, :]) & 15
    in_maps = []
    for c in range(N_CORES):
        j0, j1 = c * IC_SHARD, (c + 1) * IC_SHARD
        s_c = scales[:, j0:j1].astype(np.float32)  # [G, ic]
        # sx[rt*128 + p, j] = s[rt*8 + p//16, j]
        sx = np.repeat(s_c, 16, axis=0)  # [512, ic]
        z2b = np.empty((G_FULL + 1, IC_SHARD), dtype=np.float32)
        z2b[:G_FULL] = -(zp[:, j0:j1].astype(np.float32) * s_c)
        z2b[G_FULL] = bias[j0:j1]
        in_maps.append(
            {
                "qweight": np.ascontiguousarray(qweight[:, j0:j1]),
                "sx": np.ascontiguousarray(sx),
                "z2b": z2b.astype(ml_dtypes.bfloat16),
                "x": x,
                **consts,
            }
        )
    return in_maps


def kernel(input, qweight, qzeros, scales, bias):
    """Full-problem entry point: shard, run on 8 cores, gather."""
    from concourse.bass_utils import run_bass_kernel_spmd

    nc = bacc.Bacc("TRN2", target_bir_lowering=False, debug=False)
    build(nc)
    nc.compile()

    in_maps = make_in_maps(input, qweight, qzeros, scales, bias)
    res = run_bass_kernel_spmd(nc, in_maps, list(range(N_CORES)))
    outs = [np.asarray(res.results[c]["out"], dtype=np.float32) for c in range(N_CORES)]
    return np.concatenate(outs, axis=1)
